# revision 7
# baseline (speedup 1.0000x reference)
"""Trainium2 Bass kernel for nn_MultiHeadMinkUnet (gnn_message_passing).

Strategy: data-parallel over voxels (8192 per core, 8 cores).
 - Stage A: entropy gate (fp32, voxel-major) + 1x1 convs (channel-major
   matmuls on bf16 DMA-transposed feats) -> x rows (bf16, 128-ch padded)
 - AllGather x -> full 65536-row table per core
 - Conv1/Conv2: dma_gather (transpose mode, int16 sign-wrapped indices
   against a mid-table base) -> channel-major gathered tiles -> PE matmuls
   accumulating out^T[96,256] in PSUM -> fused BN+ReLU on ACT
 - AllGather y between convs
 - Heads: voxel-major matmuls with x2^T stationary; clip projector
   channel-major (clip1) then voxel-major (clip2)
"""

import functools
import numpy as np
import ml_dtypes

N = 65536
NC = 8
NS = N // NC          # 8192 voxels per core
K = 27
CT = 256              # conv tile voxels
NCT = NS // CT        # 32 conv tiles
GPAD = 128            # throwaway tail gather indices (sem-skew workaround)
NIP = K * CT + GPAD   # 7040 indices per gather call
IDXW = NIP // 16      # 440 wrapped idx columns per call
ST = 512              # stage-A tile voxels
NST = NS // ST        # 16 stage-A tiles
EPS_BN = 1e-5
LOG_P = float(np.log(101.0))

BF16 = ml_dtypes.bfloat16


def _build_nc():
    import concourse.bass as bass
    import concourse.bacc as bacc
    import concourse.mybir as mybir
    import concourse.tile as tile
    from concourse.masks import make_identity

    dt = mybir.dt
    AF = mybir.ActivationFunctionType
    ALU = mybir.AluOpType

    nc = bacc.Bacc("TRN2", target_bir_lowering=False, debug=False, num_devices=NC)

    # ---- I/O ----
    feats_s = nc.dram_tensor("feats_s", [NS, 105], dt.float32, kind="ExternalInput")
    idx1 = nc.dram_tensor("idx1", [128, NCT * IDXW], dt.int16, kind="ExternalInput")
    idx2 = nc.dram_tensor("idx2", [128, NCT * IDXW], dt.int16, kind="ExternalInput")
    w1flat = nc.dram_tensor("w1flat", [K * 128, 96], dt.bfloat16, kind="ExternalInput")
    w2flat = nc.dram_tensor("w2flat", [K * 128, 96], dt.bfloat16, kind="ExternalInput")
    wrgb = nc.dram_tensor("wrgb", [4, 32], dt.bfloat16, kind="ExternalInput")
    wimg = nc.dram_tensor("wimg", [128, 32], dt.bfloat16, kind="ExternalInput")
    sc64 = nc.dram_tensor("sc64", [64, 2], dt.float32, kind="ExternalInput")
    se1 = nc.dram_tensor("se1", [96, 2], dt.float32, kind="ExternalInput")
    se2 = nc.dram_tensor("se2", [96, 2], dt.float32, kind="ExternalInput")
    headsw = nc.dram_tensor("headsw", [96, 119], dt.bfloat16, kind="ExternalInput")
    wclip1 = nc.dram_tensor("wclip1", [96, 256], dt.bfloat16, kind="ExternalInput")
    sclip = nc.dram_tensor("sclip", [256, 2], dt.float32, kind="ExternalInput")
    wclip2 = nc.dram_tensor("wclip2", [256, 512], dt.bfloat16, kind="ExternalInput")
    bclip2 = nc.dram_tensor("bclip2", [1, 512], dt.float32, kind="ExternalInput")

    oluo = nc.dram_tensor("oluo", [NS, 119], dt.float32, kind="ExternalOutput")
    ox2 = nc.dram_tensor("ox2", [NS, 96], dt.float32, kind="ExternalOutput")
    oclip = nc.dram_tensor("oclip", [NS, 512], dt.float32, kind="ExternalOutput")

    # ---- internal DRAM ----
    x_loc = nc.dram_tensor("x_loc", [NS, 128], dt.bfloat16)
    y_loc = nc.dram_tensor("y_loc", [NS, 128], dt.bfloat16)
    x_full = nc.dram_tensor("x_full", [N, 128], dt.bfloat16, addr_space="Shared")
    y_full = nc.dram_tensor("y_full", [N, 128], dt.bfloat16, addr_space="Shared")

    groups = [list(range(NC))]

    with tile.TileContext(nc) as tc:
        with tc.tile_pool(name="glob", bufs=1) as gp:
            ident_f = gp.tile([128, 128], dt.float32)
            make_identity(nc, ident_f[:])
            ident_b = gp.tile([128, 128], dt.bfloat16)
            nc.vector.tensor_copy(out=ident_b[:], in_=ident_f[:])
            ones32 = gp.tile([1, 32], dt.float32)
            nc.vector.memset(ones32[:], 1.0)
            ones128 = gp.tile([1, 128], dt.float32)
            nc.vector.memset(ones128[:], 1.0)
            eps_b = gp.tile([128, 1], dt.float32)
            nc.vector.memset(eps_b[:], 1e-12)

            W1 = gp.tile([128, K * 96], dt.bfloat16)
            nc.sync.dma_start(out=W1[:].rearrange("p (k m) -> p k m", k=K), in_=w1flat[:].rearrange("(k p) m -> p k m", p=128))
            W2 = gp.tile([128, K * 96], dt.bfloat16)
            nc.sync.dma_start(out=W2[:].rearrange("p (k m) -> p k m", k=K), in_=w2flat[:].rearrange("(k p) m -> p k m", p=128))
            wrgb_sb = gp.tile([4, 32], dt.bfloat16)
            nc.sync.dma_start(out=wrgb_sb[:], in_=wrgb[:])
            wimg_sb = gp.tile([128, 32], dt.bfloat16)
            nc.sync.dma_start(out=wimg_sb[:], in_=wimg[:])
            sc64_sb = gp.tile([64, 2], dt.float32)
            nc.sync.dma_start(out=sc64_sb[:], in_=sc64[:])
            se1_sb = gp.tile([96, 2], dt.float32)
            nc.sync.dma_start(out=se1_sb[:], in_=se1[:])
            se2_sb = gp.tile([96, 2], dt.float32)
            nc.sync.dma_start(out=se2_sb[:], in_=se2[:])
            headsw_sb = gp.tile([96, 119], dt.bfloat16)
            nc.sync.dma_start(out=headsw_sb[:], in_=headsw[:])
            wc1_sb = gp.tile([96, 256], dt.bfloat16)
            nc.sync.dma_start(out=wc1_sb[:], in_=wclip1[:])
            sclip_sb = gp.tile([128, 4], dt.float32)  # [128, (j, {s,c})] j=0,1
            nc.sync.dma_start(out=sclip_sb[:].rearrange("p (j a) -> p j a", j=2), in_=sclip[:].rearrange("(j p) a -> p j a", p=128))
            wc2_sb = gp.tile([128, 2 * 512], dt.bfloat16)
            nc.sync.dma_start(out=wc2_sb[:].rearrange("p (j n) -> p j n", j=2), in_=wclip2[:].rearrange("(j p) n -> p j n", p=128))
            b2row = gp.tile([1, 512], dt.float32)
            nc.sync.dma_start(out=b2row[:], in_=bclip2[:])

            x2T_res = gp.tile([96, NS], dt.bfloat16)
            hT_res0 = gp.tile([128, NS], dt.bfloat16)
            hT_res1 = gp.tile([128, NS], dt.bfloat16)
            b2b_sb = gp.tile([128, 512], dt.float32)

            # broadcast clip2 bias to 128 partitions via ones outer product
            with tc.tile_pool(name="pre_ps", bufs=1, space="PSUM") as pp:
                ps_b2 = pp.tile([128, 512], dt.float32, space="PSUM")
                nc.tensor.matmul(out=ps_b2[:], lhsT=ones128[:], rhs=b2row[:],
                                 start=True, stop=True)
                nc.vector.tensor_copy(out=b2b_sb[:], in_=ps_b2[:])

            # ================= Stage A =================
            with tc.tile_pool(name="sa", bufs=2) as sp, \
                 tc.tile_pool(name="sa_ps", bufs=2, space="PSUM") as spp:
                for b in range(NST):
                    ft = sp.tile([128, 4, 105], dt.float32, tag="ft")
                    nc.sync.dma_start(
                        out=ft[:],
                        in_=feats_s[b * ST:(b + 1) * ST, :].rearrange("(g p) c -> p g c", p=128))
                    plp = sp.tile([128, 4, 101], dt.float32, tag="plp")
                    nc.scalar.activation(out=plp[:], in_=ft[:, :, 4:105], func=AF.Ln,
                                         bias=eps_b[:])
                    nc.vector.tensor_tensor(out=plp[:], in0=plp[:], in1=ft[:, :, 4:105],
                                            op=ALU.mult)
                    hw_t = sp.tile([128, 4], dt.float32, tag="hw")
                    nc.vector.tensor_reduce(out=hw_t[:], in_=plp[:],
                                            axis=mybir.AxisListType.X, op=ALU.add)
                    # w = 1 - (-H)/LOG_P ; hw currently holds sum(p*ln p) = -H
                    nc.vector.tensor_scalar(out=hw_t[:], in0=hw_t[:],
                                            scalar1=1.0 / LOG_P, scalar2=1.0,
                                            op0=ALU.mult, op1=ALU.add)
                    ps_wt = spp.tile([1, 512], dt.float32, tag="wt", space="PSUM")
                    for g in range(4):
                        nc.tensor.transpose(out=ps_wt[:, g * 128:(g + 1) * 128],
                                            in_=hw_t[:, g:g + 1], identity=ident_f[:])
                    wt_sb = sp.tile([1, 512], dt.float32, tag="wts")
                    nc.vector.tensor_copy(out=wt_sb[:], in_=ps_wt[:])
                    ps_wb = spp.tile([32, 512], dt.float32, tag="wb", space="PSUM")
                    nc.tensor.matmul(out=ps_wb[:], lhsT=ones32[:], rhs=wt_sb[:],
                                     start=True, stop=True)
                    wb_sb = sp.tile([32, 512], dt.float32, tag="wbs")
                    nc.scalar.activation(out=wb_sb[:], in_=ps_wb[:], func=AF.Copy)
                    # bf16 feats + transpose
                    fb = sp.tile([128, 4, 128], dt.bfloat16, tag="fb")
                    nc.gpsimd.memset(fb[:, :, 105:128], 0)
                    nc.vector.tensor_copy(out=fb[:, :, 0:105], in_=ft[:])
                    fT = sp.tile([128, 512], dt.bfloat16, tag="fT")
                    for g in range(4):
                        nc.sync.dma_start(out=fT[:, g * 128:(g + 1) * 128],
                                          in_=fb[:, g, :], transpose=True)
                    ps_x = spp.tile([64, 512], dt.float32, tag="x", space="PSUM")
                    nc.tensor.matmul(out=ps_x[0:32, :], lhsT=wrgb_sb[:], rhs=fT[0:4, :],
                                     start=True, stop=True, tile_position=(0, 0))
                    nc.tensor.matmul(out=ps_x[32:64, :], lhsT=wimg_sb[:], rhs=fT[:],
                                     start=True, stop=True, tile_position=(0, 32))
                    nc.vector.tensor_tensor(out=ps_x[32:64, :], in0=ps_x[32:64, :],
                                            in1=wb_sb[:], op=ALU.mult)
                    xT = sp.tile([64, 512], dt.bfloat16, tag="xT")
                    nc.scalar.activation(out=xT[:], in_=ps_x[:], func=AF.Relu,
                                         scale=sc64_sb[:, 0:1], bias=sc64_sb[:, 1:2])
                    ps_xr = spp.tile([128, 256], dt.bfloat16, tag="xr", space="PSUM")
                    for g in range(4):
                        nc.tensor.transpose(out=ps_xr[:, g * 64:(g + 1) * 64],
                                            in_=xT[:, g * 128:(g + 1) * 128],
                                            identity=ident_b[0:64, 0:64])
                    xs = sp.tile([128, 4, 128], dt.bfloat16, tag="xs")
                    nc.gpsimd.memset(xs[:, :, 64:128], 0)
                    nc.vector.tensor_copy(
                        out=xs[:, :, 0:64],
                        in_=ps_xr[:].rearrange("p (g c) -> p g c", g=4))
                    nc.sync.dma_start(
                        out=x_loc[b * ST:(b + 1) * ST, :].rearrange("(g p) c -> p g c", p=128),
                        in_=xs[:])

            nc.gpsimd.collective_compute(
                "AllGather", ALU.bypass, replica_groups=groups,
                ins=[x_loc[:]], outs=[x_full[:]])

            # ================= Conv1 / Conv2 =================
            for conv in (1, 2):
                src = x_full if conv == 1 else y_full
                idx_t = idx1 if conv == 1 else idx2
                Wsb = W1 if conv == 1 else W2
                sesb = se1_sb if conv == 1 else se2_sb
                with tc.tile_pool(name=f"cv{conv}", bufs=2) as cp, \
                     tc.tile_pool(name=f"cv{conv}_ps", bufs=2, space="PSUM") as cpp:
                    for t in range(NCT):
                        it = cp.tile([128, IDXW], dt.int16, tag="it")
                        nc.sync.dma_start(out=it[:], in_=idx_t[:, t * IDXW:(t + 1) * IDXW])
                        gt = cp.tile([128, NIP], dt.bfloat16, tag="gt")
                        nc.gpsimd.dma_gather(
                            out_ap=gt[:].rearrange("p (a n) -> p a n", a=1),
                            in_ap=src[N // 2:, :], idxs_ap=it[:],
                            num_idxs=NIP, num_idxs_reg=NIP,
                            elem_size=128, transpose=True, single_packet=False)
                        ps_o = cpp.tile([96, 256], dt.float32, tag="o", space="PSUM")
                        for k in range(K):
                            nc.tensor.matmul(out=ps_o[:],
                                             lhsT=Wsb[:, k * 96:(k + 1) * 96],
                                             rhs=gt[:, k * CT:(k + 1) * CT],
                                             start=(k == 0), stop=(k == K - 1))
                        if conv == 1:
                            yT = cp.tile([96, 256], dt.bfloat16, tag="yT")
                            nc.scalar.activation(out=yT[:], in_=ps_o[:], func=AF.Relu,
                                                 scale=sesb[:, 0:1], bias=sesb[:, 1:2])
                            ps_yr = cpp.tile([128, 192], dt.bfloat16, tag="yr", space="PSUM")
                            for g in range(2):
                                nc.tensor.transpose(out=ps_yr[:, g * 96:(g + 1) * 96],
                                                    in_=yT[:, g * 128:(g + 1) * 128],
                                                    identity=ident_b[0:96, 0:96])
                            ys = cp.tile([128, 2, 128], dt.bfloat16, tag="ys")
                            nc.gpsimd.memset(ys[:, :, 96:128], 0)
                            nc.vector.tensor_copy(
                                out=ys[:, :, 0:96],
                                in_=ps_yr[:].rearrange("p (g c) -> p g c", g=2))
                            nc.sync.dma_start(
                                out=y_loc[t * CT:(t + 1) * CT, :].rearrange("(g p) c -> p g c", p=128),
                                in_=ys[:])
                        else:
                            x2Tf = cp.tile([96, 256], dt.float32, tag="x2Tf")
                            nc.scalar.activation(out=x2Tf[:], in_=ps_o[:], func=AF.Relu,
                                                 scale=sesb[:, 0:1], bias=sesb[:, 1:2])
                            nc.vector.tensor_copy(out=x2T_res[:, t * CT:(t + 1) * CT],
                                                  in_=x2Tf[:])
                            ps_x2r = cpp.tile([128, 192], dt.float32, tag="yr", space="PSUM")
                            for g in range(2):
                                nc.tensor.transpose(out=ps_x2r[:, g * 96:(g + 1) * 96],
                                                    in_=x2Tf[:, g * 128:(g + 1) * 128],
                                                    identity=ident_f[0:96, 0:96])
                            x2s = cp.tile([128, 2, 96], dt.float32, tag="x2s")
                            nc.vector.tensor_copy(
                                out=x2s[:],
                                in_=ps_x2r[:].rearrange("p (g c) -> p g c", g=2))
                            nc.sync.dma_start(
                                out=ox2[t * CT:(t + 1) * CT, :].rearrange("(g p) c -> p g c", p=128),
                                in_=x2s[:])
                if conv == 1:
                    nc.gpsimd.collective_compute(
                        "AllGather", ALU.bypass, replica_groups=groups,
                        ins=[y_loc[:]], outs=[y_full[:]])

            # ================= clip1 (channel-major) =================
            with tc.tile_pool(name="c1_ps", bufs=2, space="PSUM") as hpp:
                for b in range(NST):
                    for j in range(2):
                        ps_h = hpp.tile([128, 512], dt.float32, tag=f"h{j}", space="PSUM")
                        nc.tensor.matmul(out=ps_h[:],
                                         lhsT=wc1_sb[:, j * 128:(j + 1) * 128],
                                         rhs=x2T_res[:, b * ST:(b + 1) * ST],
                                         start=True, stop=True)
                        hT = hT_res0 if j == 0 else hT_res1
                        nc.scalar.activation(out=hT[:, b * ST:(b + 1) * ST], in_=ps_h[:],
                                             func=AF.Relu,
                                             scale=sclip_sb[:, 2 * j:2 * j + 1],
                                             bias=sclip_sb[:, 2 * j + 1:2 * j + 2])

            # ================= heads + clip2 (voxel-major) =================
            with tc.tile_pool(name="hd", bufs=3) as hp, \
                 tc.tile_pool(name="hd_ps", bufs=2, space="PSUM") as dpp:
                for t in range(NS // 128):
                    sl = slice(t * 128, (t + 1) * 128)
                    ps_s = dpp.tile([128, 119], dt.float32, tag="small", space="PSUM")
                    nc.tensor.matmul(out=ps_s[:], lhsT=x2T_res[:, sl], rhs=headsw_sb[:],
                                     start=True, stop=True)
                    so = hp.tile([128, 119], dt.float32, tag="so")
                    nc.scalar.activation(out=so[:], in_=ps_s[:], func=AF.Copy)
                    nc.sync.dma_start(out=oluo[sl, :], in_=so[:])
                    ps_c = dpp.tile([128, 512], dt.float32, tag="clip", space="PSUM")
                    for j in range(2):
                        hT = hT_res0 if j == 0 else hT_res1
                        nc.tensor.matmul(out=ps_c[:], lhsT=hT[:, sl],
                                         rhs=wc2_sb[:, j * 512:(j + 1) * 512],
                                         start=(j == 0), stop=(j == 1))
                    co = hp.tile([128, 512], dt.float32, tag="co")
                    nc.vector.tensor_tensor(out=co[:], in0=ps_c[:], in1=b2b_sb[:],
                                            op=ALU.add)
                    nc.sync.dma_start(out=oclip[sl, :], in_=co[:])

    nc.compile()
    return nc


@functools.lru_cache(maxsize=1)
def _get_runtime():
    import jax
    import numpy as np
    from jax.sharding import Mesh, PartitionSpec, NamedSharding
    from jax.experimental.shard_map import shard_map
    import concourse.mybir as mybir
    from concourse import bass2jax
    from concourse.bass2jax import _bass_exec_p, install_neuronx_cc_hook

    install_neuronx_cc_hook()
    nc = _build_nc()

    partition_name = nc.partition_id_tensor.name if nc.partition_id_tensor else None
    in_names, out_names, out_avals, zero_outs = [], [], [], []
    for alloc in nc.m.functions[0].allocations:
        if not isinstance(alloc, mybir.MemoryLocationSet):
            continue
        name = alloc.memorylocations[0].name
        if alloc.kind == "ExternalInput":
            if name != partition_name:
                in_names.append(name)
        elif alloc.kind == "ExternalOutput":
            out_names.append(name)
            out_avals.append(jax.core.ShapedArray(
                tuple(alloc.tensor_shape), mybir.dt.np(alloc.dtype)))
            zero_outs.append(np.zeros(tuple(alloc.tensor_shape), mybir.dt.np(alloc.dtype)))
    all_in_names = list(in_names) + out_names
    if partition_name is not None:
        all_in_names.append(partition_name)

    def _body(*args):
        operands = list(args)
        if partition_name is not None:
            operands.append(bass2jax.partition_id_tensor())
        outs = _bass_exec_p.bind(
            *operands,
            out_avals=tuple(out_avals),
            in_names=tuple(all_in_names),
            out_names=tuple(out_names),
            lowering_input_output_aliases=(),
            sim_require_finite=True,
            sim_require_nnan=True,
            nc=nc,
        )
        return tuple(outs)

    devices = jax.devices()[:NC]
    mesh = Mesh(np.asarray(devices), ("core",))
    in_specs = (PartitionSpec("core"),) * (len(in_names) + len(out_names))
    out_specs = (PartitionSpec("core"),) * len(out_names)
    fn = jax.jit(shard_map(_body, mesh=mesh, in_specs=in_specs,
                           out_specs=out_specs, check_rep=False),
                 keep_unused=True)
    sh = NamedSharding(mesh, PartitionSpec("core"))
    return fn, in_names, out_names, out_avals, zero_outs, sh


def _fold_bn(bnp, w_bias=None):
    """bn params [4, C] (gamma, beta, mean, var) -> (scale, bias) fp32.
    If w_bias given, the conv bias is folded in: relu(s*(z+b0) ... ) ==
    relu(s*z + (s*b0 + c))."""
    g, b, m, v = bnp[0], bnp[1], bnp[2], bnp[3]
    s = g / np.sqrt(v + EPS_BN)
    c = b - s * m
    if w_bias is not None:
        c = c + s * w_bias
    return s.astype(np.float32), c.astype(np.float32)


def _prep_idx(nbr_shard):
    """nbr [NS, K] int32 global indices -> wrapped int16 idx tensor
    [128, NCT*IDXW] (sign-wrap vs mid-table base)."""
    out = np.empty((NCT, NIP), np.int64)
    for t in range(NCT):
        blk = nbr_shard[t * CT:(t + 1) * CT, :].T  # [K, CT] k-major
        out[t, :K * CT] = blk.reshape(-1)
        out[t, K * CT:] = N // 2  # pad -> idx 0 (valid row)
    w = (out - N // 2).astype(np.int16)            # sign-wrap
    w = w.reshape(NCT, IDXW, 16).transpose(2, 0, 1).reshape(16, NCT * IDXW)
    return np.tile(w, (8, 1))                      # replicate to 128 partitions


def kernel(feats, nbr1, nbr2, w_rgb, b_rgb, bn_rgb, w_img, b_img, bn_img,
           w_e1, w_e2, bn_e1, bn_e2, w_lab, w_unlab, w_over,
           w_clip1, b_clip1, bn_clip, w_clip2, b_clip2):
    import jax

    feats = np.asarray(feats, np.float32)
    nbr1 = np.asarray(nbr1); nbr2 = np.asarray(nbr2)

    # ---- host-side weight prep (tiny tensors only) ----
    s_rgb, c_rgb = _fold_bn(np.asarray(bn_rgb, np.float32), np.asarray(b_rgb, np.float32))
    s_img, c_img = _fold_bn(np.asarray(bn_img, np.float32), np.asarray(b_img, np.float32))
    sc64 = np.stack([np.concatenate([s_rgb, s_img]),
                     np.concatenate([c_rgb, c_img])], axis=1).astype(np.float32)
    s_e1, c_e1 = _fold_bn(np.asarray(bn_e1, np.float32))
    s_e2, c_e2 = _fold_bn(np.asarray(bn_e2, np.float32))
    se1 = np.stack([s_e1, c_e1], 1).astype(np.float32)
    se2 = np.stack([s_e2, c_e2], 1).astype(np.float32)
    s_cl, c_cl = _fold_bn(np.asarray(bn_clip, np.float32), np.asarray(b_clip1, np.float32))
    sclip = np.stack([s_cl, c_cl], 1).astype(np.float32)

    def padw(w, rows):  # [cin, cout] -> [rows, cout] zero-padded bf16
        out = np.zeros((rows, w.shape[1]), BF16)
        out[:w.shape[0]] = w.astype(BF16)
        return out

    w1f = np.zeros((K * 128, 96), BF16)
    w2f = np.zeros((K * 128, 96), BF16)
    we1 = np.asarray(w_e1, np.float32)  # [K, 64, 96]
    we2 = np.asarray(w_e2, np.float32)  # [K, 96, 96]
    for k in range(K):
        w1f[k * 128:k * 128 + 64] = we1[k].astype(BF16)
        w2f[k * 128:k * 128 + 96] = we2[k].astype(BF16)

    wimg_p = np.zeros((128, 32), BF16)
    wimg_p[4:105] = np.asarray(w_img, np.float32).astype(BF16)
    wrgb_b = np.asarray(w_rgb, np.float32).astype(BF16)

    wl = np.asarray(w_lab, np.float32)       # [96, 19]
    wu = np.asarray(w_unlab, np.float32)     # [5, 96, 5]
    wo = np.asarray(w_over, np.float32)      # [5, 96, 15]
    headsw = np.concatenate(
        [wl, wu.transpose(1, 0, 2).reshape(96, 25), wo.transpose(1, 0, 2).reshape(96, 75)],
        axis=1).astype(BF16)                 # [96, 119]
    wc1 = np.asarray(w_clip1, np.float32).astype(BF16)
    wc2 = np.asarray(w_clip2, np.float32).astype(BF16)
    b2 = np.asarray(b_clip2, np.float32).reshape(1, 512)

    fn, in_names, out_names, out_avals, zero_outs, sh = _get_runtime()

    # ---- per-core inputs ----
    per_core = []
    for c in range(NC):
        sl = slice(c * NS, (c + 1) * NS)
        per_core.append({
            "feats_s": feats[sl],
            "idx1": _prep_idx(nbr1[sl]),
            "idx2": _prep_idx(nbr2[sl]),
            "w1flat": w1f, "w2flat": w2f,
            "wrgb": wrgb_b, "wimg": wimg_p,
            "sc64": sc64, "se1": se1, "se2": se2,
            "headsw": headsw, "wclip1": wc1, "sclip": sclip,
            "wclip2": wc2, "bclip2": b2,
        })

    args = []
    for name in in_names:
        args.append(jax.device_put(
            np.concatenate([per_core[c][name] for c in range(NC)], axis=0), sh))
    for z in zero_outs:
        args.append(jax.device_put(
            np.zeros((NC * z.shape[0], *z.shape[1:]), z.dtype), sh))
    outs = fn(*args)
    jax.block_until_ready(outs)

    res = {name: np.asarray(outs[i]) for i, name in enumerate(out_names)}
    oluo = res["oluo"]          # [N, 119]
    x2 = res["ox2"]             # [N, 96]
    clip = res["oclip"]         # [N, 512]
    lab = np.ascontiguousarray(oluo[:, 0:19])
    unlab = np.ascontiguousarray(oluo[:, 19:44].reshape(N, 5, 5).transpose(1, 0, 2))
    over = np.ascontiguousarray(oluo[:, 44:119].reshape(N, 5, 15).transpose(1, 0, 2))
    return (lab, unlab, over, x2, clip)


# revision 8
# speedup vs baseline: 1.3890x; 1.3890x over previous
"""Trainium2 Bass kernel for nn_MultiHeadMinkUnet (gnn_message_passing).

Strategy: data-parallel over voxels (8192 per core, 8 cores).
 - Stage A: entropy gate (fp32, voxel-major) + 1x1 convs (channel-major
   matmuls on bf16 DMA-transposed feats) -> x rows (bf16, 128-ch padded)
 - AllGather x -> full 65536-row table per core
 - Conv1/Conv2: dma_gather (transpose mode, int16 sign-wrapped indices
   against a mid-table base) -> channel-major gathered tiles -> PE matmuls
   accumulating out^T[96,256] in PSUM -> fused BN+ReLU on ACT
 - AllGather y between convs
 - Heads: voxel-major matmuls with x2^T stationary; clip projector
   channel-major (clip1) then voxel-major (clip2)
"""

import functools
import numpy as np
import ml_dtypes

N = 65536
NC = 8
NS = N // NC          # 8192 voxels per core
K = 27
CT = 256              # conv tile voxels
NCT = NS // CT        # 32 conv tiles
GPAD = 128            # throwaway tail gather indices (sem-skew workaround)
NIP = K * CT + GPAD   # 7040 indices per gather call
IDXW = NIP // 16      # 440 wrapped idx columns per call
ST = 512              # stage-A tile voxels
NST = NS // ST        # 16 stage-A tiles
EPS_BN = 1e-5
LOG_P = float(np.log(101.0))

BF16 = ml_dtypes.bfloat16


def _build_nc():
    import concourse.bass as bass
    import concourse.bacc as bacc
    import concourse.mybir as mybir
    import concourse.tile as tile
    from concourse.masks import make_identity

    dt = mybir.dt
    AF = mybir.ActivationFunctionType
    ALU = mybir.AluOpType

    nc = bacc.Bacc("TRN2", target_bir_lowering=False, debug=False, num_devices=NC)

    # ---- I/O ----
    feats_s = nc.dram_tensor("feats_s", [NS, 105], dt.float32, kind="ExternalInput")
    idx1 = nc.dram_tensor("idx1", [128, NCT * IDXW], dt.int16, kind="ExternalInput")
    idx2 = nc.dram_tensor("idx2", [128, NCT * IDXW], dt.int16, kind="ExternalInput")
    w1flat = nc.dram_tensor("w1flat", [K * 128, 96], dt.bfloat16, kind="ExternalInput")
    w2flat = nc.dram_tensor("w2flat", [K * 128, 96], dt.bfloat16, kind="ExternalInput")
    wrgb = nc.dram_tensor("wrgb", [4, 32], dt.bfloat16, kind="ExternalInput")
    wimg = nc.dram_tensor("wimg", [128, 32], dt.bfloat16, kind="ExternalInput")
    sc64 = nc.dram_tensor("sc64", [64, 2], dt.float32, kind="ExternalInput")
    se1 = nc.dram_tensor("se1", [96, 2], dt.float32, kind="ExternalInput")
    se2 = nc.dram_tensor("se2", [96, 2], dt.float32, kind="ExternalInput")
    headsw = nc.dram_tensor("headsw", [96, 119], dt.bfloat16, kind="ExternalInput")
    wclip1 = nc.dram_tensor("wclip1", [96, 256], dt.bfloat16, kind="ExternalInput")
    sclip = nc.dram_tensor("sclip", [256, 2], dt.float32, kind="ExternalInput")
    wclip2 = nc.dram_tensor("wclip2", [256, 512], dt.bfloat16, kind="ExternalInput")
    bclip2 = nc.dram_tensor("bclip2", [1, 512], dt.float32, kind="ExternalInput")

    oluo = nc.dram_tensor("oluo", [NS, 119], dt.float32, kind="ExternalOutput")
    ox2 = nc.dram_tensor("ox2", [NS, 96], dt.float32, kind="ExternalOutput")
    oclip = nc.dram_tensor("oclip", [NS, 512], dt.float32, kind="ExternalOutput")

    # ---- internal DRAM ----
    x_loc = nc.dram_tensor("x_loc", [NS, 128], dt.bfloat16)
    y_loc = nc.dram_tensor("y_loc", [NS, 128], dt.bfloat16)
    x_full = nc.dram_tensor("x_full", [N, 128], dt.bfloat16, addr_space="Shared")
    y_full = nc.dram_tensor("y_full", [N, 128], dt.bfloat16, addr_space="Shared")

    groups = [list(range(NC))]

    with tile.TileContext(nc) as tc:
        with tc.tile_pool(name="glob", bufs=1) as gp:
            ident_f = gp.tile([128, 128], dt.float32)
            make_identity(nc, ident_f[:])
            ident_b = gp.tile([128, 128], dt.bfloat16)
            nc.vector.tensor_copy(out=ident_b[:], in_=ident_f[:])
            ones32 = gp.tile([1, 32], dt.float32)
            nc.vector.memset(ones32[:], 1.0)
            ones128 = gp.tile([1, 128], dt.float32)
            nc.vector.memset(ones128[:], 1.0)
            eps_b = gp.tile([128, 1], dt.float32)
            nc.vector.memset(eps_b[:], 1e-12)

            W1 = gp.tile([128, K * 96], dt.bfloat16)
            nc.sync.dma_start(out=W1[:].rearrange("p (k m) -> p k m", k=K), in_=w1flat[:].rearrange("(k p) m -> p k m", p=128))
            W2 = gp.tile([128, K * 96], dt.bfloat16)
            nc.sync.dma_start(out=W2[:].rearrange("p (k m) -> p k m", k=K), in_=w2flat[:].rearrange("(k p) m -> p k m", p=128))
            wrgb_sb = gp.tile([4, 32], dt.bfloat16)
            nc.sync.dma_start(out=wrgb_sb[:], in_=wrgb[:])
            wimg_sb = gp.tile([128, 32], dt.bfloat16)
            nc.sync.dma_start(out=wimg_sb[:], in_=wimg[:])
            sc64_sb = gp.tile([64, 2], dt.float32)
            nc.sync.dma_start(out=sc64_sb[:], in_=sc64[:])
            se1_sb = gp.tile([96, 2], dt.float32)
            nc.sync.dma_start(out=se1_sb[:], in_=se1[:])
            se2_sb = gp.tile([96, 2], dt.float32)
            nc.sync.dma_start(out=se2_sb[:], in_=se2[:])
            headsw_sb = gp.tile([96, 119], dt.bfloat16)
            nc.sync.dma_start(out=headsw_sb[:], in_=headsw[:])
            wc1_sb = gp.tile([96, 256], dt.bfloat16)
            nc.sync.dma_start(out=wc1_sb[:], in_=wclip1[:])
            sclip_sb = gp.tile([128, 4], dt.float32)  # [128, (j, {s,c})] j=0,1
            nc.sync.dma_start(out=sclip_sb[:].rearrange("p (j a) -> p j a", j=2), in_=sclip[:].rearrange("(j p) a -> p j a", p=128))
            wc2_sb = gp.tile([128, 2 * 512], dt.bfloat16)
            nc.sync.dma_start(out=wc2_sb[:].rearrange("p (j n) -> p j n", j=2), in_=wclip2[:].rearrange("(j p) n -> p j n", p=128))
            b2row = gp.tile([1, 512], dt.float32)
            nc.sync.dma_start(out=b2row[:], in_=bclip2[:])

            x2T_res = gp.tile([96, NS], dt.bfloat16)
            hT_res0 = gp.tile([128, NS], dt.bfloat16)
            hT_res1 = gp.tile([128, NS], dt.bfloat16)
            b2b_sb = gp.tile([128, 512], dt.float32)

            # broadcast clip2 bias to 128 partitions via ones outer product
            with tc.tile_pool(name="pre_ps", bufs=1, space="PSUM") as pp:
                ps_b2 = pp.tile([128, 512], dt.float32, space="PSUM")
                nc.tensor.matmul(out=ps_b2[:], lhsT=ones128[:], rhs=b2row[:],
                                 start=True, stop=True)
                nc.vector.tensor_copy(out=b2b_sb[:], in_=ps_b2[:])

            # ================= Stage A =================
            with tc.tile_pool(name="sa", bufs=2) as sp, \
                 tc.tile_pool(name="sa_ps", bufs=2, space="PSUM") as spp:
                for b in range(NST):
                    ft = sp.tile([128, 4, 105], dt.float32, tag="ft")
                    nc.sync.dma_start(
                        out=ft[:],
                        in_=feats_s[b * ST:(b + 1) * ST, :].rearrange("(g p) c -> p g c", p=128))
                    plp = sp.tile([128, 4, 101], dt.float32, tag="plp")
                    nc.scalar.activation(out=plp[:], in_=ft[:, :, 4:105], func=AF.Ln,
                                         bias=eps_b[:])
                    nc.vector.tensor_tensor(out=plp[:], in0=plp[:], in1=ft[:, :, 4:105],
                                            op=ALU.mult)
                    hw_t = sp.tile([128, 4], dt.float32, tag="hw")
                    nc.vector.tensor_reduce(out=hw_t[:], in_=plp[:],
                                            axis=mybir.AxisListType.X, op=ALU.add)
                    # w = 1 - (-H)/LOG_P ; hw currently holds sum(p*ln p) = -H
                    nc.vector.tensor_scalar(out=hw_t[:], in0=hw_t[:],
                                            scalar1=1.0 / LOG_P, scalar2=1.0,
                                            op0=ALU.mult, op1=ALU.add)
                    ps_wt = spp.tile([1, 512], dt.float32, tag="wt", space="PSUM")
                    for g in range(4):
                        nc.tensor.transpose(out=ps_wt[:, g * 128:(g + 1) * 128],
                                            in_=hw_t[:, g:g + 1], identity=ident_f[:])
                    wt_sb = sp.tile([1, 512], dt.float32, tag="wts")
                    nc.vector.tensor_copy(out=wt_sb[:], in_=ps_wt[:])
                    ps_wb = spp.tile([32, 512], dt.float32, tag="wb", space="PSUM")
                    nc.tensor.matmul(out=ps_wb[:], lhsT=ones32[:], rhs=wt_sb[:],
                                     start=True, stop=True)
                    wb_sb = sp.tile([32, 512], dt.float32, tag="wbs")
                    nc.scalar.activation(out=wb_sb[:], in_=ps_wb[:], func=AF.Copy)
                    # bf16 feats + transpose
                    fb = sp.tile([128, 4, 128], dt.bfloat16, tag="fb")
                    nc.gpsimd.memset(fb[:, :, 105:128], 0)
                    nc.vector.tensor_copy(out=fb[:, :, 0:105], in_=ft[:])
                    fT = sp.tile([128, 512], dt.bfloat16, tag="fT")
                    for g in range(4):
                        nc.sync.dma_start(out=fT[:, g * 128:(g + 1) * 128],
                                          in_=fb[:, g, :], transpose=True)
                    ps_x = spp.tile([64, 512], dt.float32, tag="x", space="PSUM")
                    nc.tensor.matmul(out=ps_x[0:32, :], lhsT=wrgb_sb[:], rhs=fT[0:4, :],
                                     start=True, stop=True, tile_position=(0, 0))
                    nc.tensor.matmul(out=ps_x[32:64, :], lhsT=wimg_sb[:], rhs=fT[:],
                                     start=True, stop=True, tile_position=(0, 32))
                    nc.vector.tensor_tensor(out=ps_x[32:64, :], in0=ps_x[32:64, :],
                                            in1=wb_sb[:], op=ALU.mult)
                    xT = sp.tile([64, 512], dt.bfloat16, tag="xT")
                    nc.scalar.activation(out=xT[:], in_=ps_x[:], func=AF.Relu,
                                         scale=sc64_sb[:, 0:1], bias=sc64_sb[:, 1:2])
                    ps_xr = spp.tile([128, 256], dt.bfloat16, tag="xr", space="PSUM")
                    for g in range(4):
                        nc.tensor.transpose(out=ps_xr[:, g * 64:(g + 1) * 64],
                                            in_=xT[:, g * 128:(g + 1) * 128],
                                            identity=ident_b[0:64, 0:64])
                    xs = sp.tile([128, 4, 128], dt.bfloat16, tag="xs")
                    nc.gpsimd.memset(xs[:, :, 64:128], 0)
                    nc.vector.tensor_copy(
                        out=xs[:, :, 0:64],
                        in_=ps_xr[:].rearrange("p (g c) -> p g c", g=4))
                    nc.sync.dma_start(
                        out=x_loc[b * ST:(b + 1) * ST, :].rearrange("(g p) c -> p g c", p=128),
                        in_=xs[:])

            nc.gpsimd.collective_compute(
                "AllGather", ALU.bypass, replica_groups=groups,
                ins=[x_loc[:]], outs=[x_full[:]])

            # ================= Conv1 / Conv2 =================
            for conv in (1, 2):
                src = x_full if conv == 1 else y_full
                idx_t = idx1 if conv == 1 else idx2
                Wsb = W1 if conv == 1 else W2
                sesb = se1_sb if conv == 1 else se2_sb
                with tc.tile_pool(name=f"cv{conv}", bufs=2) as cp, \
                     tc.tile_pool(name=f"cv{conv}_ps", bufs=2, space="PSUM") as cpp:
                    for t in range(NCT):
                        it = cp.tile([128, IDXW], dt.int16, tag="it")
                        nc.sync.dma_start(out=it[:], in_=idx_t[:, t * IDXW:(t + 1) * IDXW])
                        gt = cp.tile([128, NIP], dt.bfloat16, tag="gt")
                        nc.gpsimd.dma_gather(
                            out_ap=gt[:].rearrange("p (a n) -> p a n", a=1),
                            in_ap=src[N // 2:, :], idxs_ap=it[:],
                            num_idxs=NIP, num_idxs_reg=NIP,
                            elem_size=128, transpose=True, single_packet=False)
                        ps_o = cpp.tile([96, 256], dt.float32, tag="o", space="PSUM")
                        for k in range(K):
                            nc.tensor.matmul(out=ps_o[:],
                                             lhsT=Wsb[:, k * 96:(k + 1) * 96],
                                             rhs=gt[:, k * CT:(k + 1) * CT],
                                             start=(k == 0), stop=(k == K - 1))
                        if conv == 1:
                            yT = cp.tile([96, 256], dt.bfloat16, tag="yT")
                            nc.scalar.activation(out=yT[:], in_=ps_o[:], func=AF.Relu,
                                                 scale=sesb[:, 0:1], bias=sesb[:, 1:2])
                            ps_yr = cpp.tile([128, 192], dt.bfloat16, tag="yr", space="PSUM")
                            for g in range(2):
                                nc.tensor.transpose(out=ps_yr[:, g * 96:(g + 1) * 96],
                                                    in_=yT[:, g * 128:(g + 1) * 128],
                                                    identity=ident_b[0:96, 0:96])
                            ys = cp.tile([128, 2, 128], dt.bfloat16, tag="ys")
                            nc.gpsimd.memset(ys[:, :, 96:128], 0)
                            nc.vector.tensor_copy(
                                out=ys[:, :, 0:96],
                                in_=ps_yr[:].rearrange("p (g c) -> p g c", g=2))
                            nc.sync.dma_start(
                                out=y_loc[t * CT:(t + 1) * CT, :].rearrange("(g p) c -> p g c", p=128),
                                in_=ys[:])
                        else:
                            x2Tf = cp.tile([96, 256], dt.float32, tag="x2Tf")
                            nc.scalar.activation(out=x2Tf[:], in_=ps_o[:], func=AF.Relu,
                                                 scale=sesb[:, 0:1], bias=sesb[:, 1:2])
                            nc.vector.tensor_copy(out=x2T_res[:, t * CT:(t + 1) * CT],
                                                  in_=x2Tf[:])
                            ps_x2r = cpp.tile([128, 192], dt.float32, tag="yr", space="PSUM")
                            for g in range(2):
                                nc.tensor.transpose(out=ps_x2r[:, g * 96:(g + 1) * 96],
                                                    in_=x2Tf[:, g * 128:(g + 1) * 128],
                                                    identity=ident_f[0:96, 0:96])
                            x2s = cp.tile([128, 2, 96], dt.float32, tag="x2s")
                            nc.vector.tensor_copy(
                                out=x2s[:],
                                in_=ps_x2r[:].rearrange("p (g c) -> p g c", g=2))
                            nc.sync.dma_start(
                                out=ox2[t * CT:(t + 1) * CT, :].rearrange("(g p) c -> p g c", p=128),
                                in_=x2s[:])
                if conv == 1:
                    nc.gpsimd.collective_compute(
                        "AllGather", ALU.bypass, replica_groups=groups,
                        ins=[y_loc[:]], outs=[y_full[:]])

            # ================= clip1 (channel-major) =================
            with tc.tile_pool(name="c1_ps", bufs=2, space="PSUM") as hpp:
                for b in range(NST):
                    for j in range(2):
                        ps_h = hpp.tile([128, 512], dt.float32, tag=f"h{j}", space="PSUM")
                        nc.tensor.matmul(out=ps_h[:],
                                         lhsT=wc1_sb[:, j * 128:(j + 1) * 128],
                                         rhs=x2T_res[:, b * ST:(b + 1) * ST],
                                         start=True, stop=True)
                        hT = hT_res0 if j == 0 else hT_res1
                        nc.scalar.activation(out=hT[:, b * ST:(b + 1) * ST], in_=ps_h[:],
                                             func=AF.Relu,
                                             scale=sclip_sb[:, 2 * j:2 * j + 1],
                                             bias=sclip_sb[:, 2 * j + 1:2 * j + 2])

            # ================= heads + clip2 (voxel-major) =================
            with tc.tile_pool(name="hd", bufs=3) as hp, \
                 tc.tile_pool(name="hd_ps", bufs=2, space="PSUM") as dpp:
                for t in range(NS // 128):
                    sl = slice(t * 128, (t + 1) * 128)
                    ps_s = dpp.tile([128, 119], dt.float32, tag="small", space="PSUM")
                    nc.tensor.matmul(out=ps_s[:], lhsT=x2T_res[:, sl], rhs=headsw_sb[:],
                                     start=True, stop=True)
                    so = hp.tile([128, 119], dt.float32, tag="so")
                    nc.scalar.activation(out=so[:], in_=ps_s[:], func=AF.Copy)
                    nc.sync.dma_start(out=oluo[sl, :], in_=so[:])
                    ps_c = dpp.tile([128, 512], dt.float32, tag="clip", space="PSUM")
                    for j in range(2):
                        hT = hT_res0 if j == 0 else hT_res1
                        nc.tensor.matmul(out=ps_c[:], lhsT=hT[:, sl],
                                         rhs=wc2_sb[:, j * 512:(j + 1) * 512],
                                         start=(j == 0), stop=(j == 1))
                    co = hp.tile([128, 512], dt.float32, tag="co")
                    nc.vector.tensor_tensor(out=co[:], in0=ps_c[:], in1=b2b_sb[:],
                                            op=ALU.add)
                    nc.sync.dma_start(out=oclip[sl, :], in_=co[:])

    nc.compile()
    return nc


@functools.lru_cache(maxsize=1)
def _get_runtime():
    import jax
    import numpy as np
    from jax.sharding import Mesh, PartitionSpec, NamedSharding
    from jax.experimental.shard_map import shard_map
    import concourse.mybir as mybir
    from concourse import bass2jax
    from concourse.bass2jax import _bass_exec_p, install_neuronx_cc_hook

    install_neuronx_cc_hook()
    nc = _build_nc()

    partition_name = nc.partition_id_tensor.name if nc.partition_id_tensor else None
    in_names, out_names, out_avals, zero_outs = [], [], [], []
    for alloc in nc.m.functions[0].allocations:
        if not isinstance(alloc, mybir.MemoryLocationSet):
            continue
        name = alloc.memorylocations[0].name
        if alloc.kind == "ExternalInput":
            if name != partition_name:
                in_names.append(name)
        elif alloc.kind == "ExternalOutput":
            out_names.append(name)
            out_avals.append(jax.core.ShapedArray(
                tuple(alloc.tensor_shape), mybir.dt.np(alloc.dtype)))
            zero_outs.append(np.zeros(tuple(alloc.tensor_shape), mybir.dt.np(alloc.dtype)))
    all_in_names = list(in_names) + out_names
    if partition_name is not None:
        all_in_names.append(partition_name)

    def _body(*args):
        operands = list(args)
        if partition_name is not None:
            operands.append(bass2jax.partition_id_tensor())
        outs = _bass_exec_p.bind(
            *operands,
            out_avals=tuple(out_avals),
            in_names=tuple(all_in_names),
            out_names=tuple(out_names),
            lowering_input_output_aliases=(),
            sim_require_finite=True,
            sim_require_nnan=True,
            nc=nc,
        )
        return tuple(outs)

    devices = jax.devices()[:NC]
    mesh = Mesh(np.asarray(devices), ("core",))
    in_specs = (PartitionSpec("core"),) * (len(in_names) + len(out_names))
    out_specs = (PartitionSpec("core"),) * len(out_names)
    fn = jax.jit(shard_map(_body, mesh=mesh, in_specs=in_specs,
                           out_specs=out_specs, check_rep=False),
                 keep_unused=True)
    sh = NamedSharding(mesh, PartitionSpec("core"))
    dev_zeros = [jax.device_put(np.zeros((NC * z.shape[0], *z.shape[1:]), z.dtype), sh)
                 for z in zero_outs]
    jax.block_until_ready(dev_zeros)
    return fn, in_names, out_names, out_avals, dev_zeros, sh


def _fold_bn(bnp, w_bias=None):
    """bn params [4, C] (gamma, beta, mean, var) -> (scale, bias) fp32.
    If w_bias given, the conv bias is folded in: relu(s*(z+b0) ... ) ==
    relu(s*z + (s*b0 + c))."""
    g, b, m, v = bnp[0], bnp[1], bnp[2], bnp[3]
    s = g / np.sqrt(v + EPS_BN)
    c = b - s * m
    if w_bias is not None:
        c = c + s * w_bias
    return s.astype(np.float32), c.astype(np.float32)


def _prep_idx(nbr_shard):
    """nbr [NS, K] int32 global indices -> wrapped int16 idx tensor
    [128, NCT*IDXW] (sign-wrap vs mid-table base)."""
    out = np.empty((NCT, NIP), np.int64)
    for t in range(NCT):
        blk = nbr_shard[t * CT:(t + 1) * CT, :].T  # [K, CT] k-major
        out[t, :K * CT] = blk.reshape(-1)
        out[t, K * CT:] = N // 2  # pad -> idx 0 (valid row)
    w = (out - N // 2).astype(np.int16)            # sign-wrap
    w = w.reshape(NCT, IDXW, 16).transpose(2, 0, 1).reshape(16, NCT * IDXW)
    return np.tile(w, (8, 1))                      # replicate to 128 partitions


def kernel(feats, nbr1, nbr2, w_rgb, b_rgb, bn_rgb, w_img, b_img, bn_img,
           w_e1, w_e2, bn_e1, bn_e2, w_lab, w_unlab, w_over,
           w_clip1, b_clip1, bn_clip, w_clip2, b_clip2):
    import jax

    feats = np.asarray(feats, np.float32)
    nbr1 = np.asarray(nbr1); nbr2 = np.asarray(nbr2)

    # ---- host-side weight prep (tiny tensors only) ----
    s_rgb, c_rgb = _fold_bn(np.asarray(bn_rgb, np.float32), np.asarray(b_rgb, np.float32))
    s_img, c_img = _fold_bn(np.asarray(bn_img, np.float32), np.asarray(b_img, np.float32))
    sc64 = np.stack([np.concatenate([s_rgb, s_img]),
                     np.concatenate([c_rgb, c_img])], axis=1).astype(np.float32)
    s_e1, c_e1 = _fold_bn(np.asarray(bn_e1, np.float32))
    s_e2, c_e2 = _fold_bn(np.asarray(bn_e2, np.float32))
    se1 = np.stack([s_e1, c_e1], 1).astype(np.float32)
    se2 = np.stack([s_e2, c_e2], 1).astype(np.float32)
    s_cl, c_cl = _fold_bn(np.asarray(bn_clip, np.float32), np.asarray(b_clip1, np.float32))
    sclip = np.stack([s_cl, c_cl], 1).astype(np.float32)

    def padw(w, rows):  # [cin, cout] -> [rows, cout] zero-padded bf16
        out = np.zeros((rows, w.shape[1]), BF16)
        out[:w.shape[0]] = w.astype(BF16)
        return out

    w1f = np.zeros((K * 128, 96), BF16)
    w2f = np.zeros((K * 128, 96), BF16)
    we1 = np.asarray(w_e1, np.float32)  # [K, 64, 96]
    we2 = np.asarray(w_e2, np.float32)  # [K, 96, 96]
    for k in range(K):
        w1f[k * 128:k * 128 + 64] = we1[k].astype(BF16)
        w2f[k * 128:k * 128 + 96] = we2[k].astype(BF16)

    wimg_p = np.zeros((128, 32), BF16)
    wimg_p[4:105] = np.asarray(w_img, np.float32).astype(BF16)
    wrgb_b = np.asarray(w_rgb, np.float32).astype(BF16)

    wl = np.asarray(w_lab, np.float32)       # [96, 19]
    wu = np.asarray(w_unlab, np.float32)     # [5, 96, 5]
    wo = np.asarray(w_over, np.float32)      # [5, 96, 15]
    headsw = np.concatenate(
        [wl, wu.transpose(1, 0, 2).reshape(96, 25), wo.transpose(1, 0, 2).reshape(96, 75)],
        axis=1).astype(BF16)                 # [96, 119]
    wc1 = np.asarray(w_clip1, np.float32).astype(BF16)
    wc2 = np.asarray(w_clip2, np.float32).astype(BF16)
    b2 = np.asarray(b_clip2, np.float32).reshape(1, 512)

    fn, in_names, out_names, out_avals, dev_zeros, sh = _get_runtime()

    # ---- per-core inputs ----
    per_core = []
    for c in range(NC):
        sl = slice(c * NS, (c + 1) * NS)
        per_core.append({
            "feats_s": feats[sl],
            "idx1": _prep_idx(nbr1[sl]),
            "idx2": _prep_idx(nbr2[sl]),
            "w1flat": w1f, "w2flat": w2f,
            "wrgb": wrgb_b, "wimg": wimg_p,
            "sc64": sc64, "se1": se1, "se2": se2,
            "headsw": headsw, "wclip1": wc1, "sclip": sclip,
            "wclip2": wc2, "bclip2": b2,
        })

    import time as _time
    _t0 = _time.time()
    args = []
    for name in in_names:
        args.append(jax.device_put(
            np.concatenate([per_core[c][name] for c in range(NC)], axis=0), sh))
    args.extend(dev_zeros)
    jax.block_until_ready(args)
    _t1 = _time.time()
    outs = fn(*args)
    jax.block_until_ready(outs)
    _t2 = _time.time()

    res = {name: np.asarray(outs[i]) for i, name in enumerate(out_names)}
    _t3 = _time.time()
    import os
    if os.environ.get("KERNEL_TIMING"):
        print(f"[kernel] h2d={_t1-_t0:.3f}s exec={_t2-_t1:.3f}s d2h={_t3-_t2:.3f}s")
    oluo = res["oluo"]          # [N, 119]
    x2 = res["ox2"]             # [N, 96]
    clip = res["oclip"]         # [N, 512]
    lab = np.ascontiguousarray(oluo[:, 0:19])
    unlab = np.ascontiguousarray(oluo[:, 19:44].reshape(N, 5, 5).transpose(1, 0, 2))
    over = np.ascontiguousarray(oluo[:, 44:119].reshape(N, 5, 15).transpose(1, 0, 2))
    return (lab, unlab, over, x2, clip)


# revision 10
# speedup vs baseline: 3.7647x; 2.7104x over previous
"""Trainium2 Bass kernel for nn_MultiHeadMinkUnet (gnn_message_passing).

Strategy: data-parallel over voxels (8192 per core, 8 cores).
 - Stage A: entropy gate (fp32, voxel-major) + 1x1 convs (channel-major
   matmuls on bf16 DMA-transposed feats) -> x rows (bf16, 128-ch padded)
 - AllGather x -> full 65536-row table per core
 - Conv1/Conv2: dma_gather (transpose mode, int16 sign-wrapped indices
   against a mid-table base) -> channel-major gathered tiles -> PE matmuls
   accumulating out^T[96,256] in PSUM -> fused BN+ReLU on ACT
 - AllGather y between convs
 - Heads: voxel-major matmuls with x2^T stationary; clip projector
   channel-major (clip1) then voxel-major (clip2)
"""

import functools
import numpy as np
import ml_dtypes

N = 65536
NC = 8
NS = N // NC          # 8192 voxels per core
K = 27
CT = 256              # conv tile voxels
NCT = NS // CT        # 32 conv tiles
GPAD = 128            # throwaway tail gather indices (sem-skew workaround)
NIP = K * CT + GPAD   # 7040 indices per gather call
IDXW = NIP // 16      # 440 wrapped idx columns per call
ST = 512              # stage-A tile voxels
NST = NS // ST        # 16 stage-A tiles
EPS_BN = 1e-5
LOG_P = float(np.log(101.0))

BF16 = np.float16


def _build_nc():
    import concourse.bass as bass
    import concourse.bacc as bacc
    import concourse.mybir as mybir
    import concourse.tile as tile
    from concourse.masks import make_identity

    dt = mybir.dt
    AF = mybir.ActivationFunctionType
    ALU = mybir.AluOpType

    nc = bacc.Bacc("TRN2", target_bir_lowering=False, debug=False, num_devices=NC)

    # ---- I/O ----
    feats_s = nc.dram_tensor("feats_s", [NS, 105], dt.float32, kind="ExternalInput")
    idx1 = nc.dram_tensor("idx1", [16, NCT * IDXW], dt.int16, kind="ExternalInput")
    idx2 = nc.dram_tensor("idx2", [16, NCT * IDXW], dt.int16, kind="ExternalInput")
    w1flat = nc.dram_tensor("w1flat", [K * 128, 96], dt.float16, kind="ExternalInput")
    w2flat = nc.dram_tensor("w2flat", [K * 128, 96], dt.float16, kind="ExternalInput")
    wrgb = nc.dram_tensor("wrgb", [4, 32], dt.float16, kind="ExternalInput")
    wimg = nc.dram_tensor("wimg", [128, 32], dt.float16, kind="ExternalInput")
    sc64 = nc.dram_tensor("sc64", [64, 2], dt.float32, kind="ExternalInput")
    se1 = nc.dram_tensor("se1", [96, 2], dt.float32, kind="ExternalInput")
    se2 = nc.dram_tensor("se2", [96, 2], dt.float32, kind="ExternalInput")
    headsw = nc.dram_tensor("headsw", [96, 119], dt.float16, kind="ExternalInput")
    wclip1 = nc.dram_tensor("wclip1", [96, 256], dt.float16, kind="ExternalInput")
    sclip = nc.dram_tensor("sclip", [256, 2], dt.float32, kind="ExternalInput")
    wclip2 = nc.dram_tensor("wclip2", [256, 512], dt.float16, kind="ExternalInput")
    bclip2 = nc.dram_tensor("bclip2", [1, 512], dt.float32, kind="ExternalInput")

    oluo = nc.dram_tensor("oluo", [NS, 119], dt.float32, kind="ExternalOutput")
    ox2 = nc.dram_tensor("ox2", [NS, 96], dt.float32, kind="ExternalOutput")
    oclip = nc.dram_tensor("oclip", [NS, 512], dt.float32, kind="ExternalOutput")

    # ---- internal DRAM ----
    x_loc = nc.dram_tensor("x_loc", [NS, 128], dt.float16)
    y_loc = nc.dram_tensor("y_loc", [NS, 128], dt.float16)
    x_full = nc.dram_tensor("x_full", [N, 128], dt.float16, addr_space="Shared")
    y_full = nc.dram_tensor("y_full", [N, 128], dt.float16, addr_space="Shared")

    groups = [list(range(NC))]

    with tile.TileContext(nc) as tc:
        with tc.tile_pool(name="glob", bufs=1) as gp:
            ident_f = gp.tile([128, 128], dt.float32)
            make_identity(nc, ident_f[:])
            ident_b = gp.tile([128, 128], dt.float16)
            nc.vector.tensor_copy(out=ident_b[:], in_=ident_f[:])
            ones32 = gp.tile([1, 32], dt.float32)
            nc.vector.memset(ones32[:], 1.0)
            ones128 = gp.tile([1, 128], dt.float32)
            nc.vector.memset(ones128[:], 1.0)
            eps_b = gp.tile([128, 1], dt.float32)
            nc.vector.memset(eps_b[:], 1e-12)

            W1 = gp.tile([128, K * 96], dt.float16)
            nc.sync.dma_start(out=W1[:].rearrange("p (k m) -> p k m", k=K), in_=w1flat[:].rearrange("(k p) m -> p k m", p=128))
            W2 = gp.tile([128, K * 96], dt.float16)
            nc.sync.dma_start(out=W2[:].rearrange("p (k m) -> p k m", k=K), in_=w2flat[:].rearrange("(k p) m -> p k m", p=128))
            wrgb_sb = gp.tile([4, 32], dt.float16)
            nc.sync.dma_start(out=wrgb_sb[:], in_=wrgb[:])
            wimg_sb = gp.tile([128, 32], dt.float16)
            nc.sync.dma_start(out=wimg_sb[:], in_=wimg[:])
            sc64_sb = gp.tile([64, 2], dt.float32)
            nc.sync.dma_start(out=sc64_sb[:], in_=sc64[:])
            se1_sb = gp.tile([96, 2], dt.float32)
            nc.sync.dma_start(out=se1_sb[:], in_=se1[:])
            se2_sb = gp.tile([96, 2], dt.float32)
            nc.sync.dma_start(out=se2_sb[:], in_=se2[:])
            headsw_sb = gp.tile([96, 119], dt.float16)
            nc.sync.dma_start(out=headsw_sb[:], in_=headsw[:])
            wc1_sb = gp.tile([96, 256], dt.float16)
            nc.sync.dma_start(out=wc1_sb[:], in_=wclip1[:])
            sclip_sb = gp.tile([128, 4], dt.float32)  # [128, (j, {s,c})] j=0,1
            nc.sync.dma_start(out=sclip_sb[:].rearrange("p (j a) -> p j a", j=2), in_=sclip[:].rearrange("(j p) a -> p j a", p=128))
            wc2_sb = gp.tile([128, 2 * 512], dt.float16)
            nc.sync.dma_start(out=wc2_sb[:].rearrange("p (j n) -> p j n", j=2), in_=wclip2[:].rearrange("(j p) n -> p j n", p=128))
            b2row = gp.tile([1, 512], dt.float32)
            nc.sync.dma_start(out=b2row[:], in_=bclip2[:])

            idx1_res = gp.tile([128, NCT * IDXW], dt.int16)
            idx2_res = gp.tile([128, NCT * IDXW], dt.int16)
            for r in range(8):
                nc.sync.dma_start(out=idx1_res[16 * r:16 * (r + 1), :], in_=idx1[:])
                nc.sync.dma_start(out=idx2_res[16 * r:16 * (r + 1), :], in_=idx2[:])
            x2T_res = gp.tile([96, NS], dt.float16)
            hT_res0 = gp.tile([128, NS], dt.float16)
            hT_res1 = gp.tile([128, NS], dt.float16)
            b2b_sb = gp.tile([128, 512], dt.float32)

            # broadcast clip2 bias to 128 partitions via ones outer product
            with tc.tile_pool(name="pre_ps", bufs=1, space="PSUM") as pp:
                ps_b2 = pp.tile([128, 512], dt.float32, space="PSUM")
                nc.tensor.matmul(out=ps_b2[:], lhsT=ones128[:], rhs=b2row[:],
                                 start=True, stop=True)
                nc.vector.tensor_copy(out=b2b_sb[:], in_=ps_b2[:])

            # ================= Stage A =================
            with tc.tile_pool(name="sa", bufs=2) as sp, \
                 tc.tile_pool(name="sa_ps", bufs=2, space="PSUM") as spp:
                for b in range(NST):
                    ft = sp.tile([128, 4, 105], dt.float32, tag="ft")
                    nc.sync.dma_start(
                        out=ft[:],
                        in_=feats_s[b * ST:(b + 1) * ST, :].rearrange("(g p) c -> p g c", p=128))
                    plp = sp.tile([128, 4, 101], dt.float32, tag="plp")
                    nc.scalar.activation(out=plp[:], in_=ft[:, :, 4:105], func=AF.Ln,
                                         bias=eps_b[:])
                    nc.vector.tensor_tensor(out=plp[:], in0=plp[:], in1=ft[:, :, 4:105],
                                            op=ALU.mult)
                    hw_t = sp.tile([128, 4], dt.float32, tag="hw")
                    nc.vector.tensor_reduce(out=hw_t[:], in_=plp[:],
                                            axis=mybir.AxisListType.X, op=ALU.add)
                    # w = 1 - (-H)/LOG_P ; hw currently holds sum(p*ln p) = -H
                    nc.vector.tensor_scalar(out=hw_t[:], in0=hw_t[:],
                                            scalar1=1.0 / LOG_P, scalar2=1.0,
                                            op0=ALU.mult, op1=ALU.add)
                    ps_wt = spp.tile([1, 512], dt.float32, tag="wt", space="PSUM")
                    for g in range(4):
                        nc.tensor.transpose(out=ps_wt[:, g * 128:(g + 1) * 128],
                                            in_=hw_t[:, g:g + 1], identity=ident_f[:])
                    wt_sb = sp.tile([1, 512], dt.float32, tag="wts")
                    nc.vector.tensor_copy(out=wt_sb[:], in_=ps_wt[:])
                    ps_wb = spp.tile([32, 512], dt.float32, tag="wb", space="PSUM")
                    nc.tensor.matmul(out=ps_wb[:], lhsT=ones32[:], rhs=wt_sb[:],
                                     start=True, stop=True)
                    wb_sb = sp.tile([32, 512], dt.float32, tag="wbs")
                    nc.scalar.activation(out=wb_sb[:], in_=ps_wb[:], func=AF.Copy)
                    # bf16 feats + transpose
                    fb = sp.tile([128, 4, 128], dt.float16, tag="fb")
                    nc.gpsimd.memset(fb[:, :, 105:128], 0)
                    nc.vector.tensor_copy(out=fb[:, :, 0:105], in_=ft[:])
                    fT = sp.tile([128, 512], dt.float16, tag="fT")
                    for g in range(4):
                        nc.sync.dma_start(out=fT[:, g * 128:(g + 1) * 128],
                                          in_=fb[:, g, :], transpose=True)
                    ps_x = spp.tile([64, 512], dt.float32, tag="x", space="PSUM")
                    nc.tensor.matmul(out=ps_x[0:32, :], lhsT=wrgb_sb[:], rhs=fT[0:4, :],
                                     start=True, stop=True, tile_position=(0, 0))
                    nc.tensor.matmul(out=ps_x[32:64, :], lhsT=wimg_sb[:], rhs=fT[:],
                                     start=True, stop=True, tile_position=(0, 32))
                    nc.vector.tensor_tensor(out=ps_x[32:64, :], in0=ps_x[32:64, :],
                                            in1=wb_sb[:], op=ALU.mult)
                    xT = sp.tile([64, 512], dt.float16, tag="xT")
                    nc.scalar.activation(out=xT[:], in_=ps_x[:], func=AF.Relu,
                                         scale=sc64_sb[:, 0:1], bias=sc64_sb[:, 1:2])
                    ps_xr = spp.tile([128, 256], dt.float16, tag="xr", space="PSUM")
                    for g in range(4):
                        nc.tensor.transpose(out=ps_xr[:, g * 64:(g + 1) * 64],
                                            in_=xT[:, g * 128:(g + 1) * 128],
                                            identity=ident_b[0:64, 0:64])
                    xs = sp.tile([128, 4, 128], dt.float16, tag="xs")
                    nc.gpsimd.memset(xs[:, :, 64:128], 0)
                    nc.vector.tensor_copy(
                        out=xs[:, :, 0:64],
                        in_=ps_xr[:].rearrange("p (g c) -> p g c", g=4))
                    nc.sync.dma_start(
                        out=x_loc[b * ST:(b + 1) * ST, :].rearrange("(g p) c -> p g c", p=128),
                        in_=xs[:])

            nc.gpsimd.collective_compute(
                "AllGather", ALU.bypass, replica_groups=groups,
                ins=[x_loc[:]], outs=[x_full[:]])

            # ================= Conv1 / Conv2 =================
            for conv in (1, 2):
                src = x_full if conv == 1 else y_full
                idx_res = idx1_res if conv == 1 else idx2_res
                Wsb = W1 if conv == 1 else W2
                sesb = se1_sb if conv == 1 else se2_sb
                with tc.tile_pool(name=f"cv{conv}", bufs=2) as cp, \
                     tc.tile_pool(name=f"cv{conv}_ps", bufs=2, space="PSUM") as cpp:
                    for t in range(NCT):
                        gt = cp.tile([128, NIP], dt.float16, tag="gt")
                        nc.gpsimd.dma_gather(
                            out_ap=gt[:].rearrange("p (a n) -> p a n", a=1),
                            in_ap=src[N // 2:, :],
                            idxs_ap=idx_res[:, t * IDXW:(t + 1) * IDXW],
                            num_idxs=NIP, num_idxs_reg=NIP,
                            elem_size=128, transpose=True, single_packet=False)
                        ps_o = cpp.tile([96, 256], dt.float32, tag="o", space="PSUM")
                        for k in range(K):
                            nc.tensor.matmul(out=ps_o[:],
                                             lhsT=Wsb[:, k * 96:(k + 1) * 96],
                                             rhs=gt[:, k * CT:(k + 1) * CT],
                                             start=(k == 0), stop=(k == K - 1))
                        if conv == 1:
                            yT = cp.tile([96, 256], dt.float16, tag="yT")
                            nc.scalar.activation(out=yT[:], in_=ps_o[:], func=AF.Relu,
                                                 scale=sesb[:, 0:1], bias=sesb[:, 1:2])
                            ps_yr = cpp.tile([128, 192], dt.float16, tag="yr", space="PSUM")
                            for g in range(2):
                                nc.tensor.transpose(out=ps_yr[:, g * 96:(g + 1) * 96],
                                                    in_=yT[:, g * 128:(g + 1) * 128],
                                                    identity=ident_b[0:96, 0:96])
                            ys = cp.tile([128, 2, 128], dt.float16, tag="ys")
                            nc.gpsimd.memset(ys[:, :, 96:128], 0)
                            nc.vector.tensor_copy(
                                out=ys[:, :, 0:96],
                                in_=ps_yr[:].rearrange("p (g c) -> p g c", g=2))
                            nc.sync.dma_start(
                                out=y_loc[t * CT:(t + 1) * CT, :].rearrange("(g p) c -> p g c", p=128),
                                in_=ys[:])
                        else:
                            x2Tf = cp.tile([96, 256], dt.float32, tag="x2Tf")
                            nc.scalar.activation(out=x2Tf[:], in_=ps_o[:], func=AF.Relu,
                                                 scale=sesb[:, 0:1], bias=sesb[:, 1:2])
                            nc.vector.tensor_copy(out=x2T_res[:, t * CT:(t + 1) * CT],
                                                  in_=x2Tf[:])
                            ps_x2r = cpp.tile([128, 192], dt.float32, tag="yr", space="PSUM")
                            for g in range(2):
                                nc.tensor.transpose(out=ps_x2r[:, g * 96:(g + 1) * 96],
                                                    in_=x2Tf[:, g * 128:(g + 1) * 128],
                                                    identity=ident_f[0:96, 0:96])
                            x2s = cp.tile([128, 2, 96], dt.float32, tag="x2s")
                            nc.vector.tensor_copy(
                                out=x2s[:],
                                in_=ps_x2r[:].rearrange("p (g c) -> p g c", g=2))
                            nc.sync.dma_start(
                                out=ox2[t * CT:(t + 1) * CT, :].rearrange("(g p) c -> p g c", p=128),
                                in_=x2s[:])
                if conv == 1:
                    nc.gpsimd.collective_compute(
                        "AllGather", ALU.bypass, replica_groups=groups,
                        ins=[y_loc[:]], outs=[y_full[:]])

            # ================= clip1 (channel-major) =================
            with tc.tile_pool(name="c1_ps", bufs=2, space="PSUM") as hpp:
                for b in range(NST):
                    for j in range(2):
                        ps_h = hpp.tile([128, 512], dt.float32, tag=f"h{j}", space="PSUM")
                        nc.tensor.matmul(out=ps_h[:],
                                         lhsT=wc1_sb[:, j * 128:(j + 1) * 128],
                                         rhs=x2T_res[:, b * ST:(b + 1) * ST],
                                         start=True, stop=True)
                        hT = hT_res0 if j == 0 else hT_res1
                        nc.scalar.activation(out=hT[:, b * ST:(b + 1) * ST], in_=ps_h[:],
                                             func=AF.Relu,
                                             scale=sclip_sb[:, 2 * j:2 * j + 1],
                                             bias=sclip_sb[:, 2 * j + 1:2 * j + 2])

            # ================= heads + clip2 (voxel-major) =================
            with tc.tile_pool(name="hd", bufs=3) as hp, \
                 tc.tile_pool(name="hd_ps", bufs=2, space="PSUM") as dpp:
                for t in range(NS // 128):
                    sl = slice(t * 128, (t + 1) * 128)
                    ps_s = dpp.tile([128, 119], dt.float32, tag="small", space="PSUM")
                    nc.tensor.matmul(out=ps_s[:], lhsT=x2T_res[:, sl], rhs=headsw_sb[:],
                                     start=True, stop=True)
                    so = hp.tile([128, 119], dt.float32, tag="so")
                    nc.scalar.activation(out=so[:], in_=ps_s[:], func=AF.Copy)
                    nc.sync.dma_start(out=oluo[sl, :], in_=so[:])
                    ps_c = dpp.tile([128, 512], dt.float32, tag="clip", space="PSUM")
                    for j in range(2):
                        hT = hT_res0 if j == 0 else hT_res1
                        nc.tensor.matmul(out=ps_c[:], lhsT=hT[:, sl],
                                         rhs=wc2_sb[:, j * 512:(j + 1) * 512],
                                         start=(j == 0), stop=(j == 1))
                    co = hp.tile([128, 512], dt.float32, tag="co")
                    nc.vector.tensor_tensor(out=co[:], in0=ps_c[:], in1=b2b_sb[:],
                                            op=ALU.add)
                    nc.sync.dma_start(out=oclip[sl, :], in_=co[:])

    nc.compile()
    return nc


@functools.lru_cache(maxsize=1)
def _get_runtime():
    import jax
    import numpy as np
    from jax.sharding import Mesh, PartitionSpec, NamedSharding
    from jax.experimental.shard_map import shard_map
    import concourse.mybir as mybir
    from concourse import bass2jax
    from concourse.bass2jax import _bass_exec_p, install_neuronx_cc_hook

    install_neuronx_cc_hook()
    nc = _build_nc()

    partition_name = nc.partition_id_tensor.name if nc.partition_id_tensor else None
    in_names, out_names, out_avals, zero_outs = [], [], [], []
    for alloc in nc.m.functions[0].allocations:
        if not isinstance(alloc, mybir.MemoryLocationSet):
            continue
        name = alloc.memorylocations[0].name
        if alloc.kind == "ExternalInput":
            if name != partition_name:
                in_names.append(name)
        elif alloc.kind == "ExternalOutput":
            out_names.append(name)
            out_avals.append(jax.core.ShapedArray(
                tuple(alloc.tensor_shape), mybir.dt.np(alloc.dtype)))
            zero_outs.append(np.zeros(tuple(alloc.tensor_shape), mybir.dt.np(alloc.dtype)))
    all_in_names = list(in_names) + out_names
    if partition_name is not None:
        all_in_names.append(partition_name)

    def _body(*args):
        operands = list(args)
        if partition_name is not None:
            operands.append(bass2jax.partition_id_tensor())
        outs = _bass_exec_p.bind(
            *operands,
            out_avals=tuple(out_avals),
            in_names=tuple(all_in_names),
            out_names=tuple(out_names),
            lowering_input_output_aliases=(),
            sim_require_finite=True,
            sim_require_nnan=True,
            nc=nc,
        )
        return tuple(outs)

    devices = jax.devices()[:NC]
    mesh = Mesh(np.asarray(devices), ("core",))
    in_specs = (PartitionSpec("core"),) * (len(in_names) + len(out_names))
    out_specs = (PartitionSpec("core"),) * len(out_names)
    fn = jax.jit(shard_map(_body, mesh=mesh, in_specs=in_specs,
                           out_specs=out_specs, check_rep=False),
                 keep_unused=True)
    sh = NamedSharding(mesh, PartitionSpec("core"))
    dev_zeros = [jax.device_put(np.zeros((NC * z.shape[0], *z.shape[1:]), z.dtype), sh)
                 for z in zero_outs]
    jax.block_until_ready(dev_zeros)
    return fn, in_names, out_names, out_avals, dev_zeros, sh


def _fold_bn(bnp, w_bias=None):
    """bn params [4, C] (gamma, beta, mean, var) -> (scale, bias) fp32.
    If w_bias given, the conv bias is folded in: relu(s*(z+b0) ... ) ==
    relu(s*z + (s*b0 + c))."""
    g, b, m, v = bnp[0], bnp[1], bnp[2], bnp[3]
    s = g / np.sqrt(v + EPS_BN)
    c = b - s * m
    if w_bias is not None:
        c = c + s * w_bias
    return s.astype(np.float32), c.astype(np.float32)


def _prep_idx(nbr_shard):
    """nbr [NS, K] int32 global indices -> wrapped int16 idx tensor
    [128, NCT*IDXW] (sign-wrap vs mid-table base)."""
    out = np.empty((NCT, NIP), np.int64)
    for t in range(NCT):
        blk = nbr_shard[t * CT:(t + 1) * CT, :].T  # [K, CT] k-major
        out[t, :K * CT] = blk.reshape(-1)
        out[t, K * CT:] = N // 2  # pad -> idx 0 (valid row)
    w = (out - N // 2).astype(np.int16)            # sign-wrap
    return w.reshape(NCT, IDXW, 16).transpose(2, 0, 1).reshape(16, NCT * IDXW)


def kernel(feats, nbr1, nbr2, w_rgb, b_rgb, bn_rgb, w_img, b_img, bn_img,
           w_e1, w_e2, bn_e1, bn_e2, w_lab, w_unlab, w_over,
           w_clip1, b_clip1, bn_clip, w_clip2, b_clip2):
    import jax

    feats = np.asarray(feats, np.float32)
    nbr1 = np.asarray(nbr1); nbr2 = np.asarray(nbr2)

    # ---- host-side weight prep (tiny tensors only) ----
    s_rgb, c_rgb = _fold_bn(np.asarray(bn_rgb, np.float32), np.asarray(b_rgb, np.float32))
    s_img, c_img = _fold_bn(np.asarray(bn_img, np.float32), np.asarray(b_img, np.float32))
    sc64 = np.stack([np.concatenate([s_rgb, s_img]),
                     np.concatenate([c_rgb, c_img])], axis=1).astype(np.float32)
    s_e1, c_e1 = _fold_bn(np.asarray(bn_e1, np.float32))
    s_e2, c_e2 = _fold_bn(np.asarray(bn_e2, np.float32))
    se1 = np.stack([s_e1, c_e1], 1).astype(np.float32)
    se2 = np.stack([s_e2, c_e2], 1).astype(np.float32)
    s_cl, c_cl = _fold_bn(np.asarray(bn_clip, np.float32), np.asarray(b_clip1, np.float32))
    sclip = np.stack([s_cl, c_cl], 1).astype(np.float32)

    def padw(w, rows):  # [cin, cout] -> [rows, cout] zero-padded bf16
        out = np.zeros((rows, w.shape[1]), BF16)
        out[:w.shape[0]] = w.astype(BF16)
        return out

    w1f = np.zeros((K * 128, 96), BF16)
    w2f = np.zeros((K * 128, 96), BF16)
    we1 = np.asarray(w_e1, np.float32)  # [K, 64, 96]
    we2 = np.asarray(w_e2, np.float32)  # [K, 96, 96]
    for k in range(K):
        w1f[k * 128:k * 128 + 64] = we1[k].astype(BF16)
        w2f[k * 128:k * 128 + 96] = we2[k].astype(BF16)

    wimg_p = np.zeros((128, 32), BF16)
    wimg_p[4:105] = np.asarray(w_img, np.float32).astype(BF16)
    wrgb_b = np.asarray(w_rgb, np.float32).astype(BF16)

    wl = np.asarray(w_lab, np.float32)       # [96, 19]
    wu = np.asarray(w_unlab, np.float32)     # [5, 96, 5]
    wo = np.asarray(w_over, np.float32)      # [5, 96, 15]
    headsw = np.concatenate(
        [wl, wu.transpose(1, 0, 2).reshape(96, 25), wo.transpose(1, 0, 2).reshape(96, 75)],
        axis=1).astype(BF16)                 # [96, 119]
    wc1 = np.asarray(w_clip1, np.float32).astype(BF16)
    wc2 = np.asarray(w_clip2, np.float32).astype(BF16)
    b2 = np.asarray(b_clip2, np.float32).reshape(1, 512)

    fn, in_names, out_names, out_avals, dev_zeros, sh = _get_runtime()

    # ---- per-core inputs ----
    per_core = []
    for c in range(NC):
        sl = slice(c * NS, (c + 1) * NS)
        per_core.append({
            "feats_s": feats[sl],
            "idx1": _prep_idx(nbr1[sl]),
            "idx2": _prep_idx(nbr2[sl]),
            "w1flat": w1f, "w2flat": w2f,
            "wrgb": wrgb_b, "wimg": wimg_p,
            "sc64": sc64, "se1": se1, "se2": se2,
            "headsw": headsw, "wclip1": wc1, "sclip": sclip,
            "wclip2": wc2, "bclip2": b2,
        })

    import time as _time
    _t0 = _time.time()
    args = []
    for name in in_names:
        args.append(jax.device_put(
            np.concatenate([per_core[c][name] for c in range(NC)], axis=0), sh))
    args.extend(dev_zeros)
    jax.block_until_ready(args)
    _t1 = _time.time()
    outs = fn(*args)
    jax.block_until_ready(outs)
    _t2 = _time.time()

    outs_np = jax.device_get(list(outs))
    res = {name: outs_np[i] for i, name in enumerate(out_names)}
    _t3 = _time.time()
    import os
    if os.environ.get("KERNEL_TIMING"):
        print(f"[kernel] h2d={_t1-_t0:.3f}s exec={_t2-_t1:.3f}s d2h={_t3-_t2:.3f}s")
    oluo = res["oluo"]          # [N, 119]
    x2 = res["ox2"]             # [N, 96]
    clip = res["oclip"]         # [N, 512]
    lab = np.ascontiguousarray(oluo[:, 0:19])
    unlab = np.ascontiguousarray(oluo[:, 19:44].reshape(N, 5, 5).transpose(1, 0, 2))
    over = np.ascontiguousarray(oluo[:, 44:119].reshape(N, 5, 15).transpose(1, 0, 2))
    return (lab, unlab, over, x2, clip)
